# revision 1
# baseline (speedup 1.0000x reference)
"""LogicLayer Trainium2 kernel: out = k0 + k1*a + k2*b + k3*a*b where
k = softmax(weights) @ OP_COEFFS, a = x[:, conn0], b = x[:, conn1].

Strategy: shard out_dim (65536) across 8 NeuronCores (8192 neurons each).
Neurons live on partitions (transposed formulation). The column gathers
become row gathers of x^T (staged host-side as fp16 [in_dim, batch], 1KB
rows) executed with GPSIMD dma_gather (SWDGE indirect DMA). Softmax->k
folding runs on device (ACT exp + DVE reduce/reciprocal). The affine
combine runs on DVE: two fused per-partition tensor_scalar affine maps
plus two fp16 tensor_tensor ops (measured rel err ~3e-4 << 2e-2).
Host reassembles the transposed fp16 shards into the [512, 65536] f32
output.
"""
import numpy as np

from concourse import bacc, mybir, tile
from concourse.bass_utils import run_bass_kernel_spmd

# Problem constants (hardcoded per contract; see module docstring).
BATCH = 512
IN_DIM = 8192
OUT_DIM = 65536
N_CORES = 8
SHARD = OUT_DIM // N_CORES  # 8192 neurons per core
P = 128
NCOL = SHARD // P  # 64 k-columns per core; neuron = col*128 + p

# Gather/compute tiling.
CH_IDX = 1024            # neurons per gather chunk (>1024 wedges SWDGE ring)
N_CHUNK = SHARD // CH_IDX  # 8
COLS_PER_CHUNK = CH_IDX // P  # 8
OUT_COLS = 8             # columns per output DMA block (= one gather chunk)
N_BLK = NCOL // OUT_COLS  # 8 output blocks per core

FP16 = mybir.dt.float16
F32 = mybir.dt.float32
I16 = mybir.dt.int16

OP_COEFFS = np.array([
    [0.0,  0.0,  0.0,  0.0],
    [0.0,  0.0,  0.0,  1.0],
    [0.0,  1.0,  0.0, -1.0],
    [0.0,  1.0,  0.0,  0.0],
    [0.0,  0.0,  1.0, -1.0],
    [0.0,  0.0,  1.0,  0.0],
    [0.0,  1.0,  1.0, -2.0],
    [0.0,  1.0,  1.0, -1.0],
    [1.0, -1.0, -1.0,  1.0],
    [1.0, -1.0, -1.0,  2.0],
    [1.0,  0.0, -1.0,  0.0],
    [1.0,  0.0, -1.0,  1.0],
    [1.0, -1.0,  0.0,  0.0],
    [1.0, -1.0,  0.0,  1.0],
    [1.0,  0.0,  0.0, -1.0],
    [1.0,  0.0,  0.0,  0.0],
], dtype=np.float32)


WORK_BUFS = 4


def build_program(n_reps: int = 1):
    """Build the per-core Bass program. n_reps>1 repeats the whole kernel
    body (for repeat-delta wall-clock timing); outputs are overwritten."""
    nc = bacc.Bacc("TRN2", target_bir_lowering=False, debug=False,
                   num_devices=N_CORES, num_swdge_queues=2)

    xt = nc.dram_tensor("xt", [IN_DIM, BATCH], FP16, kind="ExternalInput")
    w = nc.dram_tensor("w", [P, NCOL * 16], F32, kind="ExternalInput")
    coef = nc.dram_tensor("coef", [P, 64], F32, kind="ExternalInput")
    idxa = nc.dram_tensor("idxa", [P, N_CHUNK * (CH_IDX // 16)], I16, kind="ExternalInput")
    idxb = nc.dram_tensor("idxb", [P, N_CHUNK * (CH_IDX // 16)], I16, kind="ExternalInput")
    out = nc.dram_tensor("out", [N_BLK, P, OUT_COLS, BATCH], FP16, kind="ExternalOutput")

    with tile.TileContext(nc) as tc:
        with tc.tile_pool(name="const", bufs=1) as cpool, \
             tc.tile_pool(name="work", bufs=WORK_BUFS) as pool:
            for _rep in range(n_reps):
                # ---- k-coefficient computation: k = softmax(w) @ OP_COEFFS
                w_sb = cpool.tile([P, NCOL * 16], F32, tag="w_sb")
                coef_sb = cpool.tile([P, 64], F32, tag="coef_sb")
                nc.sync.dma_start(out=w_sb[:], in_=w[:])
                nc.sync.dma_start(out=coef_sb[:], in_=coef[:])

                e = cpool.tile([P, NCOL * 16], F32, tag="e")
                nc.scalar.activation(e[:], w_sb[:], mybir.ActivationFunctionType.Exp)
                e3 = e[:].rearrange("p (c i) -> p c i", i=16)

                s = cpool.tile([P, NCOL], F32, tag="s")
                nc.vector.tensor_reduce(out=s[:], in_=e3, axis=mybir.AxisListType.X,
                                        op=mybir.AluOpType.add)
                rs = cpool.tile([P, NCOL], F32, tag="rs")
                nc.vector.reciprocal(rs[:], s[:])

                k = []
                for cc in range(4):
                    m = cpool.tile([P, NCOL * 16], F32, tag="ktmp")
                    cb = coef_sb[:, cc * 16:(cc + 1) * 16].unsqueeze(1).broadcast_to(
                        [P, NCOL, 16])
                    nc.vector.tensor_tensor(out=m[:].rearrange("p (c i) -> p c i", i=16),
                                            in0=e3, in1=cb, op=mybir.AluOpType.mult)
                    ks = cpool.tile([P, NCOL], F32, tag=f"ksum{cc}")
                    nc.vector.tensor_reduce(out=ks[:],
                                            in_=m[:].rearrange("p (c i) -> p c i", i=16),
                                            axis=mybir.AxisListType.X,
                                            op=mybir.AluOpType.add)
                    kc = cpool.tile([P, NCOL], F32, tag=f"k{cc}")
                    nc.vector.tensor_tensor(out=kc[:], in0=ks[:], in1=rs[:],
                                            op=mybir.AluOpType.mult)
                    k.append(kc)

                # ---- main loop: gather a/b rows, affine combine, store
                ia_all = cpool.tile([P, N_CHUNK * (CH_IDX // 16)], I16, tag="ia_all")
                ib_all = cpool.tile([P, N_CHUNK * (CH_IDX // 16)], I16, tag="ib_all")
                nc.sync.dma_start(out=ia_all[:], in_=idxa[:])
                nc.sync.dma_start(out=ib_all[:], in_=idxb[:])
                for chunk in range(N_CHUNK):
                    csl = slice(chunk * (CH_IDX // 16), (chunk + 1) * (CH_IDX // 16))
                    ia = ia_all[:, csl]
                    ib = ib_all[:, csl]

                    a_t = pool.tile([P, COLS_PER_CHUNK, BATCH], FP16, tag="a")
                    b_t = pool.tile([P, COLS_PER_CHUNK, BATCH], FP16, tag="b")
                    nc.gpsimd.dma_gather(
                        out_ap=a_t[:], in_ap=xt[:], idxs_ap=ia,
                        num_idxs=CH_IDX, num_idxs_reg=CH_IDX, elem_size=BATCH,
                        queue_num=0)
                    nc.gpsimd.dma_gather(
                        out_ap=b_t[:], in_ap=xt[:], idxs_ap=ib,
                        num_idxs=CH_IDX, num_idxs_reg=CH_IDX, elem_size=BATCH,
                        queue_num=1)

                    out_t = pool.tile([P, OUT_COLS, BATCH], FP16, tag="out_t")
                    t1 = pool.tile([P, COLS_PER_CHUNK, BATCH], FP16, tag="t1")
                    t2 = pool.tile([P, COLS_PER_CHUNK, BATCH], FP16, tag="t2")
                    for g in range(COLS_PER_CHUNK):
                        col = chunk * COLS_PER_CHUNK + g
                        a_sl = a_t[:, g, :]
                        nc.vector.tensor_scalar(
                            out=t1[:, g, :], in0=a_sl,
                            scalar1=k[3][:, col:col + 1],
                            scalar2=k[2][:, col:col + 1],
                            op0=mybir.AluOpType.mult, op1=mybir.AluOpType.add)
                        nc.vector.tensor_scalar(
                            out=t2[:, g, :], in0=a_sl,
                            scalar1=k[1][:, col:col + 1],
                            scalar2=k[0][:, col:col + 1],
                            op0=mybir.AluOpType.mult, op1=mybir.AluOpType.add)
                    nc.vector.tensor_tensor(out=t1[:], in0=t1[:], in1=b_t[:],
                                            op=mybir.AluOpType.mult)
                    nc.vector.tensor_tensor(
                        out=out_t[:], in0=t1[:], in1=t2[:], op=mybir.AluOpType.add)
                    nc.sync.dma_start(out=out[chunk], in_=out_t[:])
    nc.compile()
    return nc


def make_in_maps(x, weights, connections):
    """Host-side sharding/staging. x [512,8192] f32, weights [65536,16],
    connections [65536,2] int."""
    xt = np.ascontiguousarray(x.T.astype(np.float16))  # [IN_DIM, BATCH]
    coef_dev = np.tile(OP_COEFFS.T.reshape(1, 64), (P, 1)).astype(np.float32)
    coef_dev = np.ascontiguousarray(coef_dev)

    in_maps = []
    for c in range(N_CORES):
        base = c * SHARD
        w_shard = weights[base:base + SHARD]  # [SHARD, 16]
        # w_dev[p, col*16+i] = w_shard[col*128+p, i]
        w_dev = np.ascontiguousarray(
            w_shard.reshape(NCOL, P, 16).transpose(1, 0, 2).reshape(P, NCOL * 16)
        ).astype(np.float32)

        conn = connections[base:base + SHARD].astype(np.int16)  # [SHARD, 2]
        idx = []
        for j in range(2):
            # chunk slot s -> neuron base + chunk*CH_IDX + s;
            # wrapped layout [16, CH_IDX//16]: w[i, t] = idx[t*16+i], tiled to 128.
            arr = conn[:, j].reshape(N_CHUNK, CH_IDX // 16, 16).transpose(0, 2, 1)
            arr = np.tile(arr, (1, 8, 1))  # [N_CHUNK, 128, CH_IDX//16]
            idx.append(np.ascontiguousarray(
                arr.transpose(1, 0, 2).reshape(P, -1)))  # [128, N_CHUNK*CH//16]
        in_maps.append({
            "xt": xt, "w": w_dev, "coef": coef_dev,
            "idxa": idx[0], "idxb": idx[1],
        })
    return in_maps


def assemble_output(results):
    """results: per-core dicts with 'out' [N_BLK, P, OUT_COLS, BATCH] fp16.
    DRAM [blk, p, g, :] holds neuron (blk*OUT_COLS + g)*128 + p."""
    shards = []
    for c in range(N_CORES):
        o = results[c]["out"]  # [4, 128, 16, 512]
        shards.append(o.transpose(0, 2, 1, 3).reshape(SHARD, BATCH))
    full = np.concatenate(shards, axis=0)  # [OUT_DIM, BATCH]
    return np.ascontiguousarray(full.T.astype(np.float32))


_CACHED_NC = None


def kernel(x, weights, connections):
    global _CACHED_NC
    if _CACHED_NC is None:
        _CACHED_NC = build_program()
    in_maps = make_in_maps(np.asarray(x), np.asarray(weights),
                           np.asarray(connections))
    last_err = None
    for _attempt in range(3):
        try:
            res = run_bass_kernel_spmd(_CACHED_NC, in_maps, list(range(N_CORES)))
            return assemble_output(res.results)
        except Exception as e:  # transient device wedge: retry
            last_err = e
    raise last_err


if __name__ == "__main__":
    rng = np.random.default_rng(0)
    x = rng.random((BATCH, IN_DIM), dtype=np.float32)
    weights = (rng.standard_normal((OUT_DIM, 16)) * 0.1).astype(np.float32)
    connections = rng.integers(0, IN_DIM, size=(OUT_DIM, 2), dtype=np.int64)
    out = kernel(x, weights, connections)
    print("out", out.shape, out.dtype)

